# revision 3
# baseline (speedup 1.0000x reference)
"""Causal multi-head self-attention on 8 Trainium2 NeuronCores.

Problem: x[2,2048,1024], 16 heads, dk=64, causal softmax, fp32 in/out.

Sharding (data + tensor parallel per the hint): core c handles batch
b = c//4 and head group g = c%4 (4 heads = 256 feature cols). wq/wk/wv
column-sharded, wo row-sharded; each core returns a [D, S] partial of
out^T for its batch; the host sums the 4 partials per batch.

Design (against the TimelineSim cost model, where a matmul costs its
output free-size in streamed columns; K/M and weight loads are free):
  - AV is flipped: per (head, q-block) one accumulation chain over the
    causal k-tiles with lhsT = e-block [k=128, q=128] and rhs =
    v_aug [k, 65] (v plus a ones column) -> av [q, 65] streams 65 cols
    per block instead of 128; col 64 is the softmax denominator.
  - Normalization is a per-partition reciprocal+multiply in the [q, dk]
    orientation; attn^T for the wo projection comes from PE
    transpose-mode matmuls (identity rhs), odd heads land at partition
    base 64 directly.
  - e tiles live per-unit in SBUF (exp writes them once; all q-block
    chains read them), so PSUM pressure stays low: scores double-buffer
    4 banks, av chains 2 banks, filler+transposes 2 banks.
  - The causal diagonal 128x128 block gets the additive staircase mask
    (-240*(k-q) for k>q) via one extra matmul; exp underflows those
    entries to exact zeros.
  - Emission: per (pair, q-chunk) units pipeline scores_j (PE) ->
    exp_j (ACT) -> chain for the just-completed q-block (PE), with
    projection / wo pieces (512-col granularity, own psum pool) paced
    into each j-step by its ACT-vs-PE deficit; during the initial
    x-chunk DMA stream, six projection pieces chase the arrivals using
    the otherwise-idle attention banks.
"""

import sys

import numpy as np

if "/opt/trn_rl_repo" not in sys.path:
    sys.path.insert(0, "/opt/trn_rl_repo")

B, S, D, H, DK = 2, 2048, 1024, 16, 64
HPC = 4            # heads per core
GW = HPC * DK      # 256
NCORES = 8
QC = 1024          # q-chunk width (2 chunks)
KT = 128           # k-tile
NQB = QC // KT     # q-blocks per chunk (8)

_CACHE = {}


def _build_nc(reps=1):
    import concourse.bacc as bacc
    import concourse.tile as tile
    import concourse.bass as bass
    from concourse import mybir

    f32 = mybir.dt.float32
    bf = mybir.dt.bfloat16
    Exp = mybir.ActivationFunctionType.Exp
    PSUM = bass.MemorySpace.PSUM

    nc = bacc.Bacc(
        "TRN2",
        target_bir_lowering=False,
        debug=False,
        enable_asserts=False,
        num_devices=NCORES,
    )

    xT_d = nc.dram_tensor("xT", [D, S], bf, kind="ExternalInput")
    wq_d = nc.dram_tensor("wq", [D, GW], bf, kind="ExternalInput")
    wk_d = nc.dram_tensor("wk", [D, GW], bf, kind="ExternalInput")
    wv_d = nc.dram_tensor("wv", [D, GW], bf, kind="ExternalInput")
    wo_d = nc.dram_tensor("wo", [GW, D], bf, kind="ExternalInput")
    ident_d = nc.dram_tensor("ident", [128, 128], bf, kind="ExternalInput")
    stA_d = nc.dram_tensor("stairA", [128, 128], bf, kind="ExternalInput")
    stB_d = nc.dram_tensor("stairB", [128, 128], bf, kind="ExternalInput")
    # bf16 output partials: halves the output DMA; the host accumulates the
    # four per-batch partials in float64, keeping rel-err ~0.5% << 2% gate.
    outT_d = nc.dram_tensor("outT", [D, S], bf, kind="ExternalOutput")

    KC = D // 128  # 8 contraction chunks for the projections
    EW = [4608, 12800]  # flat e-tile width per q-chunk (sum of causal w_j)

    with tile.TileContext(nc) as tc:
        with (
            tc.tile_pool(name="weights", bufs=1) as wpool,
            tc.tile_pool(name="acts", bufs=1) as apool,
            tc.tile_pool(name="epool", bufs=2) as epool,
            tc.tile_pool(name="small", bufs=4) as spool,
            tc.tile_pool(name="outp", bufs=3) as opool,
            tc.tile_pool(name="pssc", bufs=2, space=PSUM) as pssc,
            tc.tile_pool(name="psav", bufs=2, space=PSUM) as psav,
            tc.tile_pool(name="psfl", bufs=2, space=PSUM) as psfl,
        ):
            ident = wpool.tile([128, 128], bf, tag="ident")
            stA = wpool.tile([128, 128], bf, tag="stA")
            stB = wpool.tile([128, 128], bf, tag="stB")
            wq_sb = wpool.tile([128, KC, GW], bf, tag="wq")
            wk_sb = wpool.tile([128, KC, GW], bf, tag="wk")
            wv_sb = wpool.tile([128, KC, GW], bf, tag="wv")
            wo_sb = wpool.tile([128, 2, D], bf, tag="wo")
            ones_sb = wpool.tile([128, 1], bf, tag="ones")

            first_rep = True
            for _rep in range(reps):  # >1 only for timing builds
                xT_sb = apool.tile([128, KC, S], bf, tag="xT", name=f"xT_sb{_rep}")
                xT_view = xT_d.ap().rearrange("(kc p) s -> p kc s", p=128)
                # Half of wq, then x chunk 0, then the rest of wq/wk split
                # between the first chunk arrivals: the first projection
                # matmul starts ~4us in, and the pieces chase the stream.
                wq_view = wq_d.ap().rearrange("(kc p) m -> p kc m", p=128)
                wk_view = wk_d.ap().rearrange("(kc p) m -> p kc m", p=128)
                if first_rep:
                    nc.sync.dma_start(wq_sb[:, 0:4, :], wq_view[:, 0:4, :])
                nc.sync.dma_start(xT_sb[:, 0, 0:1024], xT_view[:, 0, 0:1024])
                nc.sync.dma_start(xT_sb[:, 0, 1024:2048],
                                  xT_view[:, 0, 1024:2048])
                if first_rep:
                    nc.sync.dma_start(wq_sb[:, 4:8, :], wq_view[:, 4:8, :])
                    nc.sync.dma_start(wk_sb[:, 0:4, :], wk_view[:, 0:4, :])
                nc.sync.dma_start(xT_sb[:, 1, :], xT_view[:, 1, :])
                if first_rep:
                    nc.sync.dma_start(wk_sb[:, 4:8, :], wk_view[:, 4:8, :])
                for kc in range(2, KC):
                    nc.sync.dma_start(xT_sb[:, kc, :], xT_view[:, kc, :])
                if first_rep:
                    first_rep = False
                    nc.sync.dma_start(
                        wv_sb, wv_d.ap().rearrange("(kc p) m -> p kc m", p=128))
                    nc.sync.dma_start(ident, ident_d.ap())
                    nc.sync.dma_start(stA, stA_d.ap())
                    nc.sync.dma_start(stB, stB_d.ap())
                    nc.sync.dma_start(
                        wo_sb, wo_d.ap().rearrange("(f p) n -> p f n", p=128))
                    nc.gpsimd.memset(ones_sb, 1.0)

                qT_sb = apool.tile([128, 2, S], bf, tag="qT")
                kT_sb = apool.tile([128, 2, S], bf, tag="kT")
                # v with a ones column per head: [k, st, head, dk+1]
                v_sb = apool.tile([128, S // 128, HPC, DK + 1], bf, tag="v")
                attnT = apool.tile([128, 2, S], bf, tag="attnT")

                def proj_qk_piece(m, c2, name, seg, pool=None, ptag="fl"):
                    w_sb, dst = (wq_sb, qT_sb) if name == "q" else (wk_sb, kT_sb)
                    a = QC * c2 + 512 * seg
                    ps = (pool or psfl).tile([128, 512], f32, tag=ptag,
                                             name=f"pqk{name}{m}{c2}{seg}")
                    for kc in range(KC):
                        nc.tensor.matmul(
                            ps,
                            lhsT=w_sb[:, kc, 128 * m:128 * (m + 1)],
                            rhs=xT_sb[:, kc, a:a + 512],
                            start=(kc == 0),
                            stop=(kc == KC - 1),
                        )
                    nc.vector.tensor_copy(dst[:, m, a:a + 512], ps)

                def proj_v(st, pool=None, ptag="fl"):
                    pool = pool or psfl
                    ps = pool.tile([128, GW], f32, tag=ptag, name=f"pv{st}")
                    for kc in range(KC):
                        nc.tensor.matmul(
                            ps,
                            lhsT=xT_sb[:, kc, 128 * st:128 * (st + 1)],
                            rhs=wv_sb[:, kc, :],
                            start=(kc == 0),
                            stop=(kc == KC - 1),
                        )
                    vdst = v_sb[:, st, :, :]
                    nc.vector.tensor_copy(
                        vdst[:, :, 0:DK],
                        ps.rearrange("p (h w) -> p h w", w=DK))
                    nc.gpsimd.memset(vdst[:, :, DK:DK + 1], 1.0)

                def wo_piece(c2, dm, seg, q=None, pool=None, ptag="fl",
                             evac=None):
                    # out-DMA issue rotates between the SP and gpsimd DGE
                    # queues (tail pieces add ACT) so the sequencer's ~1.2us
                    # per-DMA issue cost doesn't serialize the drain.
                    a = QC * c2 + 512 * seg
                    po = (pool or psfl).tile([128, 512], f32, tag=ptag,
                                             name=f"po{c2}{dm}{seg}")
                    for f in range(2):
                        nc.tensor.matmul(
                            po,
                            lhsT=wo_sb[:, f, 128 * dm:128 * (dm + 1)],
                            rhs=attnT[:, f, a:a + 512],
                            start=(f == 0),
                            stop=(f == 1),
                        )
                    ob = opool.tile([128, 512], bf, tag="ob",
                                    name=f"ob{c2}{dm}{seg}")
                    if evac == "act":
                        nc.scalar.copy(ob, po)
                    else:
                        nc.vector.tensor_copy(ob, po)
                    (q or nc.sync).dma_start(
                        outT_d.ap()[128 * dm:128 * (dm + 1), a:a + 512], ob)

                # ---------------- attention unit ----------------
                class Unit:
                    """One (head-pair mi, q-chunk c) attention unit."""

                    def __init__(self, mi, c):
                        self.mi, self.c = mi, c
                        self.q0 = QC * c
                        self.njt = (self.q0 + QC) // KT
                        self.e = epool.tile([128, 2, EW[c]], bf, tag="e",
                                            name=f"e{mi}{c}")
                        self.av4 = None
                        self.av4_idx = 4
                        self.av4_started = False
                        self.off = []
                        o = 0
                        for j in range(self.njt):
                            self.off.append(o)
                            o += QC - max(0, KT * j - self.q0)

                    def step_scores_exp(self, j):
                        """scores + exp for k-tile j. Per 512-col segment,
                        one psum tile [128, 2, seg] holds both heads (one
                        bank each) and ONE activation exps both; the
                        staircase masks the diagonal block (always the first
                        128 cols of segment 0) before exp."""
                        mi, q0 = self.mi, self.q0
                        k0 = KT * j
                        vs = max(0, k0 - q0)
                        for si, (a, b) in enumerate(
                                [(vs, 512), (512, QC)] if vs < 512
                                else [(vs, QC)]):
                            # full [128, 2, 512] tile even for narrow
                            # segments: the head stride must be exactly one
                            # PSUM bank so each head's accumulation group is
                            # bank-private.
                            ps = pssc.tile([128, 2, 512], f32, tag="sc",
                                           name=f"ps{mi}{self.c}_{j}_{si}")
                            diag_here = (k0 >= q0) and (a == vs)
                            for hh in range(2):
                                pb = 64 * hh
                                nc.tensor.matmul(
                                    ps[:, hh, 0:b - a],
                                    lhsT=kT_sb[pb:pb + DK, mi, k0:k0 + KT],
                                    rhs=qT_sb[pb:pb + DK, mi, q0 + a:q0 + b],
                                    start=True,
                                    stop=not diag_here,
                                )
                                if diag_here:
                                    nc.tensor.matmul(
                                        ps[:, hh, 0:KT],
                                        lhsT=stA,
                                        rhs=stB,
                                        start=False,
                                        stop=True,
                                    )
                            o = self.off[j] + a - vs
                            nc.scalar.activation(
                                self.e[:, :, o:o + b - a],
                                ps[:, :, 0:b - a], Exp, scale=0.125)

                    def chain_qb(self, qb, dma_tp=False, act_norm=False):
                        """q-block qb is complete after exp j=qg: run its AV
                        chains, normalize, transpose into attnT. Four chains
                        (2 q-blocks x 2 heads) pack into one PSUM bank as a
                        single accumulation group: the first matmul's start
                        pending-zeroes the bank, later chains' first writes
                        consume it, the last chain's last matmul stops.
                        Transpose into attnT via XBAR DMA (both heads in one
                        [128,128] call, off the PE) except when latency
                        matters (dma_tp=False: PE transpose-mode)."""
                        mi, c, q0 = self.mi, self.c, self.q0
                        qg = NQB * c + qb
                        aq2 = spool.tile([128, 2, DK], bf, tag="aq2",
                                         name=f"aq2_{mi}{c}{qb}", bufs=6)
                        if not dma_tp:
                            tp = psfl.tile([128, 128], bf, tag="fl",
                                           name=f"tp{mi}{c}{qb}")
                        for hh in range(2):
                            h = 2 * mi + hh
                            if self.av4_idx == 4:
                                self.av4 = psav.tile(
                                    [128, 4, DK + 1], f32, tag="av",
                                    name=f"av4_{mi}{c}{qb}")
                                self.av4_idx = 0
                                self.av4_started = False
                            av = self.av4[:, self.av4_idx, :]
                            last_in_group = self.av4_idx == 3
                            self.av4_idx += 1
                            for j in range(qg + 1):
                                vs = max(0, KT * j - q0)
                                col = self.off[j] + 128 * qg - q0 - vs
                                nc.tensor.matmul(
                                    av,
                                    lhsT=self.e[:, hh, col:col + 128],
                                    rhs=v_sb[:, j, h, :],
                                    start=not self.av4_started,
                                    stop=(last_in_group and j == qg),
                                )
                                self.av4_started = True
                            rden = spool.tile([128, 1], f32, tag=f"rden{hh}",
                                              name=f"rden{mi}{c}{qb}{hh}")
                            nc.vector.reciprocal(rden, av[:, DK:DK + 1])
                            if act_norm:
                                nc.scalar.mul(aq2[:, hh, :], av[:, 0:DK], rden)
                            else:
                                nc.vector.tensor_scalar_mul(
                                    aq2[:, hh, :], av[:, 0:DK], rden)
                            if not dma_tp:
                                nc.tensor.transpose(
                                    tp[64 * hh:64 * (hh + 1), :],
                                    aq2[:, hh, :], ident)
                        if dma_tp:
                            nc.sync.dma_start_transpose(
                                attnT[:, mi, KT * qg:KT * (qg + 1)], aq2)
                        else:
                            nc.vector.tensor_copy(
                                attnT[:, mi, KT * qg:KT * (qg + 1)], tp)

                # ---------------- emission schedule ----------------
                import os as _os
                FILLF = float(_os.environ.get('FILLF', '1.3'))
                PE_NS = 1.0 / 2.4  # warm ns per streamed column
                proj_q, done = [], set()

                def piece(key, cols, fn):
                    proj_q.append((key, cols * PE_NS, fn))

                def fill_one():
                    if proj_q:
                        key, cost, fn = proj_q.pop(0)
                        fn()
                        done.add(key)
                        return cost
                    return None

                def fill_ns(budget):
                    while budget > 0 and proj_q:
                        budget -= fill_one()

                def need(*keys):
                    while not all(k in done for k in keys):
                        if fill_one() is None:
                            raise RuntimeError(f"missing pieces {keys}")

                def run_unit(u, needs0=(), needs8=(), inject=None,
                             dma_tp=False, act_norm=False):
                    need(*needs0)
                    u.step_scores_exp(0)
                    for j in range(u.njt):
                        vs = max(0, KT * j - u.q0)
                        w = QC - vs
                        diag = KT * j >= u.q0

                        # budget: cover the exps minus the scores work; the
                        # AV chains are dependency-blocked on exp_j, so they
                        # don't count as guaranteed PE coverage.
                        nseg = 2 if vs < 512 else 1
                        act_j = 2 * 0.833 * w + 185 * nseg
                        pe_j = PE_NS * 2 * (w + (2 * KT if diag else 0))
                        if j + 1 == NQB and needs8:
                            need(*needs8)
                        if j + 1 < u.njt:
                            u.step_scores_exp(j + 1)
                        if diag:
                            qb = j - NQB * u.c
                            need(f"v{j}")
                            u.chain_qb(qb, dma_tp=dma_tp,
                                       act_norm=act_norm)
                        if inject:
                            for fn in inject.pop(j, []):
                                fn()
                        fill_ns((act_j - pe_j) * FILLF)

                # Unit order (0,1), (1,1), (0,0), (1,0): the q-chunk-1 units
                # (long exp chains) run while projection filler is plentiful;
                # the final unit is a cheap c=0 one, so the tail after the
                # last exp is short and wo chunk 1 becomes mid-kernel filler.

                # startup: six projection pieces chase the x-chunk DMAs,
                # parked on the otherwise-idle attention psum banks.
                proj_qk_piece(0, 1, "q", 0)
                proj_qk_piece(0, 1, "q", 1)
                done.update({"q01a", "q01b"})

                def k_piece(seg):  # k(m=0) chunk 0, on the av banks
                    a = 512 * seg
                    ps = psav.tile([128, 512], f32, tag="av", name=f"pk{seg}")
                    for kc in range(KC):
                        nc.tensor.matmul(
                            ps,
                            lhsT=wk_sb[:, kc, 0:128],
                            rhs=xT_sb[:, kc, a:a + 512],
                            start=(kc == 0),
                            stop=(kc == KC - 1),
                        )
                    nc.vector.tensor_copy(kT_sb[:, 0, a:a + 512], ps)

                k_piece(0)
                k_piece(1)
                done.update({"k00a", "k00b"})
                # q00 pieces can chase the x stream (wq is resident); v
                # tiles can't (wv lands after the last x chunk).
                proj_qk_piece(0, 0, "q", 0, pool=pssc, ptag="sc")
                proj_qk_piece(0, 0, "q", 1, pool=pssc, ptag="sc")
                done.update({"q00a", "q00b"})
                piece("v0", 2048, lambda: proj_v(0))
                piece("v1", 2048, lambda: proj_v(1))

                def qk_pieces(nm, m, c2):
                    for sg, sn in ((0, "a"), (1, "b")):
                        piece(f"{nm}{m}{c2}{sn}", 4096,
                              lambda nm=nm, m=m, c2=c2, sg=sg:
                              proj_qk_piece(m, c2, nm, sg))

                for st in range(2, NQB):
                    piece(f"v{st}", 2048, lambda st=st: proj_v(st))
                qk_pieces("k", 0, 1)
                for st in range(NQB, 12):
                    piece(f"v{st}", 2048, lambda st=st: proj_v(st))
                qk_pieces("q", 1, 1)
                qk_pieces("k", 1, 0)
                for st in range(12, 16):
                    piece(f"v{st}", 2048, lambda st=st: proj_v(st))
                qk_pieces("k", 1, 1)
                qk_pieces("q", 1, 0)

                run_unit(Unit(0, 1),
                         needs0=("q01a", "q01b", "k00a", "k00b"),
                         needs8=("k01a", "k01b"))
                # wo chunk-1 seg-0 only needs q-blocks 8..11 of both c=1
                # units: inject into the tail of the second unit.
                inject = {12 + i: [
                    (lambda dm=dm: wo_piece(1, dm, 0)) for dm in (2 * i, 2 * i + 1)
                ] for i in range(4)}
                run_unit(Unit(1, 1),
                         needs0=("q11a", "q11b", "k10a", "k10b"),
                         needs8=("k11a", "k11b"),
                         inject=inject)

                for dm in range(8):
                    piece(f"wo1{dm}1", 1024,
                          lambda dm=dm: wo_piece(1, dm, 1))

                run_unit(Unit(0, 0))
                inject = {4 + i: [
                    (lambda dm=dm: wo_piece(0, dm, 0)) for dm in (2 * i, 2 * i + 1)
                ] for i in range(4)}
                run_unit(Unit(1, 0), needs0=("q10a", "q10b"), inject=inject,
                         act_norm=True)
                while fill_one() is not None:
                    pass

                # tail: the attention banks and ScalarE are idle now -
                # rotate psum pools and evac engines, and pair up the final
                # out-DMAs (two 128-row slabs per transfer) so the HWDGE +
                # DMA-semaphore pipeline drains in half the steps.
                a1 = 512  # chunk 0, segment 1
                out_pair_view = outT_d.ap().rearrange(
                    "(dmp two p) c -> dmp p two c", two=2, p=128)
                for dmp in range(4):
                    ob2 = opool.tile([128, 2, 512], bf, tag="ob2",
                                     name=f"ob2_{dmp}", bufs=3)
                    for half in range(2):
                        dm = 2 * dmp + half
                        pool, ptag = [(psav, "av"), (pssc, "sc")][dm % 2]
                        po = pool.tile([128, 512], f32, tag=ptag,
                                       name=f"po0{dm}1")
                        for f in range(2):
                            nc.tensor.matmul(
                                po,
                                lhsT=wo_sb[:, f, 128 * dm:128 * (dm + 1)],
                                rhs=attnT[:, f, a1:a1 + 512],
                                start=(f == 0),
                                stop=(f == 1),
                            )
                        if dm % 2:
                            nc.scalar.copy(ob2[:, half, :], po)
                        else:
                            nc.vector.tensor_copy(ob2[:, half, :], po)
                    nc.sync.dma_start(
                        out_pair_view[dmp, :, :, a1:a1 + 512], ob2)

    nc.compile()
    return nc


def _get_nc():
    if "nc" not in _CACHE:
        _CACHE["nc"] = _build_nc()
    return _CACHE["nc"]


def _consts():
    import ml_dtypes

    t = np.arange(128)
    ident = np.eye(128).astype(ml_dtypes.bfloat16)
    stA = (t[:, None] <= t[None, :]).astype(ml_dtypes.bfloat16)
    stB = np.where(t[:, None] > t[None, :], -240.0, 0.0).astype(ml_dtypes.bfloat16)
    return ident, stA, stB


def _make_in_maps(x, wq, wk, wv, wo):
    import ml_dtypes

    bf = ml_dtypes.bfloat16
    ident, stA, stB = _consts()
    x = np.asarray(x, np.float32)
    xTs = [np.ascontiguousarray(x[b].T).astype(bf) for b in range(B)]
    wqb = np.asarray(wq, np.float32).astype(bf)
    wkb = np.asarray(wk, np.float32).astype(bf)
    wvb = np.asarray(wv, np.float32).astype(bf)
    wob = np.asarray(wo, np.float32).astype(bf)
    in_maps = []
    for c in range(NCORES):
        b, g = divmod(c, HPC)
        cols = slice(g * GW, (g + 1) * GW)
        in_maps.append({
            "xT": xTs[b],
            "wq": np.ascontiguousarray(wqb[:, cols]),
            "wk": np.ascontiguousarray(wkb[:, cols]),
            "wv": np.ascontiguousarray(wvb[:, cols]),
            "wo": np.ascontiguousarray(wob[cols, :]),
            "ident": ident,
            "stairA": stA,
            "stairB": stB,
        })
    return in_maps


def run(x, wq, wk, wv, wo, trace=False):
    from concourse.bass_utils import run_bass_kernel_spmd

    nc = _get_nc()
    in_maps = _make_in_maps(x, wq, wk, wv, wo)
    res = run_bass_kernel_spmd(nc, in_maps, list(range(NCORES)), trace=trace)
    acc = np.zeros((B, D, S), np.float64)
    for c in range(NCORES):
        acc[c // HPC] += res.results[c]["outT"]
    out = np.ascontiguousarray(acc.transpose(0, 2, 1).astype(np.float32))
    return out, res


def kernel(x, wq, wk, wv, wo):
    out, _ = run(x, wq, wk, wv, wo, trace=False)
    return out


# revision 4
# speedup vs baseline: 1.0138x; 1.0138x over previous
"""Causal multi-head self-attention on 8 Trainium2 NeuronCores.

Problem: x[2,2048,1024], 16 heads, dk=64, causal softmax, fp32 in/out.

Sharding (data + tensor parallel per the hint): core c handles batch
b = c//4 and head group g = c%4 (4 heads = 256 feature cols). wq/wk/wv
column-sharded, wo row-sharded; each core returns a [D, S] partial of
out^T for its batch; the host sums the 4 partials per batch.

Design (against the TimelineSim cost model, where a matmul costs its
output free-size in streamed columns; K/M and weight loads are free):
  - AV is flipped: per (head, q-block) one accumulation chain over the
    causal k-tiles with lhsT = e-block [k=128, q=128] and rhs =
    v_aug [k, 65] (v plus a ones column) -> av [q, 65] streams 65 cols
    per block instead of 128; col 64 is the softmax denominator.
  - Normalization is a per-partition reciprocal+multiply in the [q, dk]
    orientation; attn^T for the wo projection comes from PE
    transpose-mode matmuls (identity rhs), odd heads land at partition
    base 64 directly.
  - e tiles live per-unit in SBUF (exp writes them once; all q-block
    chains read them), so PSUM pressure stays low: scores double-buffer
    4 banks, av chains 2 banks, filler+transposes 2 banks.
  - The causal diagonal 128x128 block gets the additive staircase mask
    (-240*(k-q) for k>q) via one extra matmul; exp underflows those
    entries to exact zeros.
  - Emission: per (pair, q-chunk) units pipeline scores_j (PE) ->
    exp_j (ACT) -> chain for the just-completed q-block (PE), with
    projection / wo pieces (512-col granularity, own psum pool) paced
    into each j-step by its ACT-vs-PE deficit; during the initial
    x-chunk DMA stream, six projection pieces chase the arrivals using
    the otherwise-idle attention banks.
"""

import sys

import numpy as np

if "/opt/trn_rl_repo" not in sys.path:
    sys.path.insert(0, "/opt/trn_rl_repo")

B, S, D, H, DK = 2, 2048, 1024, 16, 64
HPC = 4            # heads per core
GW = HPC * DK      # 256
NCORES = 8
QC = 1024          # q-chunk width (2 chunks)
KT = 128           # k-tile
NQB = QC // KT     # q-blocks per chunk (8)

_CACHE = {}


def _build_nc(reps=1):
    import concourse.bacc as bacc
    import concourse.tile as tile
    import concourse.bass as bass
    from concourse import mybir

    f32 = mybir.dt.float32
    bf = mybir.dt.bfloat16
    Exp = mybir.ActivationFunctionType.Exp
    PSUM = bass.MemorySpace.PSUM

    nc = bacc.Bacc(
        "TRN2",
        target_bir_lowering=False,
        debug=False,
        enable_asserts=False,
        num_devices=NCORES,
    )

    xT_d = nc.dram_tensor("xT", [D, S], bf, kind="ExternalInput")
    wq_d = nc.dram_tensor("wq", [D, GW], bf, kind="ExternalInput")
    wk_d = nc.dram_tensor("wk", [D, GW], bf, kind="ExternalInput")
    wv_d = nc.dram_tensor("wv", [D, GW], bf, kind="ExternalInput")
    wo_d = nc.dram_tensor("wo", [GW, D], bf, kind="ExternalInput")
    ident_d = nc.dram_tensor("ident", [128, 128], bf, kind="ExternalInput")
    stA_d = nc.dram_tensor("stairA", [128, 128], bf, kind="ExternalInput")
    stB_d = nc.dram_tensor("stairB", [128, 128], bf, kind="ExternalInput")
    # bf16 output partials: halves the output DMA; the host accumulates the
    # four per-batch partials in float64, keeping rel-err ~0.5% << 2% gate.
    outT_d = nc.dram_tensor("outT", [D, S], bf, kind="ExternalOutput")

    KC = D // 128  # 8 contraction chunks for the projections
    EW = [4608, 12800]  # flat e-tile width per q-chunk (sum of causal w_j)

    with tile.TileContext(nc) as tc:
        with (
            tc.tile_pool(name="weights", bufs=1) as wpool,
            tc.tile_pool(name="acts", bufs=1) as apool,
            tc.tile_pool(name="epool", bufs=2) as epool,
            tc.tile_pool(name="small", bufs=4) as spool,
            tc.tile_pool(name="outp", bufs=3) as opool,
            tc.tile_pool(name="pssc", bufs=2, space=PSUM) as pssc,
            tc.tile_pool(name="psav", bufs=2, space=PSUM) as psav,
            tc.tile_pool(name="psfl", bufs=2, space=PSUM) as psfl,
        ):
            ident = wpool.tile([128, 128], bf, tag="ident")
            stA = wpool.tile([128, 128], bf, tag="stA")
            stB = wpool.tile([128, 128], bf, tag="stB")
            wq_sb = wpool.tile([128, KC, GW], bf, tag="wq")
            wk_sb = wpool.tile([128, KC, GW], bf, tag="wk")
            wv_sb = wpool.tile([128, KC, GW], bf, tag="wv")
            wo_sb = wpool.tile([128, 2, D], bf, tag="wo")
            ones_sb = wpool.tile([128, 1], bf, tag="ones")

            first_rep = True
            for _rep in range(reps):  # >1 only for timing builds
                xT_sb = apool.tile([128, KC, S], bf, tag="xT", name=f"xT_sb{_rep}")
                xT_view = xT_d.ap().rearrange("(kc p) s -> p kc s", p=128)
                # Half of wq, then x chunk 0, then the rest of wq/wk split
                # between the first chunk arrivals: the first projection
                # matmul starts ~4us in, and the pieces chase the stream.
                wq_view = wq_d.ap().rearrange("(kc p) m -> p kc m", p=128)
                wk_view = wk_d.ap().rearrange("(kc p) m -> p kc m", p=128)
                if first_rep:
                    nc.sync.dma_start(wq_sb[:, 0:4, :], wq_view[:, 0:4, :])
                nc.sync.dma_start(xT_sb[:, 0, 0:1024], xT_view[:, 0, 0:1024])
                nc.sync.dma_start(xT_sb[:, 0, 1024:2048],
                                  xT_view[:, 0, 1024:2048])
                if first_rep:
                    nc.sync.dma_start(wq_sb[:, 4:8, :], wq_view[:, 4:8, :])
                    nc.sync.dma_start(wk_sb[:, 0:4, :], wk_view[:, 0:4, :])
                nc.sync.dma_start(xT_sb[:, 1, :], xT_view[:, 1, :])
                if first_rep:
                    nc.sync.dma_start(wk_sb[:, 4:8, :], wk_view[:, 4:8, :])
                for kc in range(2, KC):
                    nc.sync.dma_start(xT_sb[:, kc, :], xT_view[:, kc, :])
                if first_rep:
                    first_rep = False
                    nc.sync.dma_start(
                        wv_sb, wv_d.ap().rearrange("(kc p) m -> p kc m", p=128))
                    nc.sync.dma_start(ident, ident_d.ap())
                    nc.sync.dma_start(stA, stA_d.ap())
                    nc.sync.dma_start(stB, stB_d.ap())
                    nc.sync.dma_start(
                        wo_sb, wo_d.ap().rearrange("(f p) n -> p f n", p=128))
                    nc.gpsimd.memset(ones_sb, 1.0)

                qT_sb = apool.tile([128, 2, S], bf, tag="qT")
                kT_sb = apool.tile([128, 2, S], bf, tag="kT")
                # v with a ones column per head: [k, st, head, dk+1]
                v_sb = apool.tile([128, S // 128, HPC, DK + 1], bf, tag="v")
                attnT = apool.tile([128, 2, S], bf, tag="attnT")

                def proj_qk_piece(m, c2, name, seg, pool=None, ptag="fl"):
                    w_sb, dst = (wq_sb, qT_sb) if name == "q" else (wk_sb, kT_sb)
                    a = QC * c2 + 512 * seg
                    ps = (pool or psfl).tile([128, 512], f32, tag=ptag,
                                             name=f"pqk{name}{m}{c2}{seg}")
                    for kc in range(KC):
                        nc.tensor.matmul(
                            ps,
                            lhsT=w_sb[:, kc, 128 * m:128 * (m + 1)],
                            rhs=xT_sb[:, kc, a:a + 512],
                            start=(kc == 0),
                            stop=(kc == KC - 1),
                        )
                    nc.vector.tensor_copy(dst[:, m, a:a + 512], ps)

                def proj_v(st, pool=None, ptag="fl"):
                    pool = pool or psfl
                    ps = pool.tile([128, GW], f32, tag=ptag, name=f"pv{st}")
                    for kc in range(KC):
                        nc.tensor.matmul(
                            ps,
                            lhsT=xT_sb[:, kc, 128 * st:128 * (st + 1)],
                            rhs=wv_sb[:, kc, :],
                            start=(kc == 0),
                            stop=(kc == KC - 1),
                        )
                    vdst = v_sb[:, st, :, :]
                    nc.vector.tensor_copy(
                        vdst[:, :, 0:DK],
                        ps.rearrange("p (h w) -> p h w", w=DK))
                    nc.gpsimd.memset(vdst[:, :, DK:DK + 1], 1.0)

                def wo_piece(c2, dm, seg, q=None, pool=None, ptag="fl",
                             evac=None):
                    # out-DMA issue rotates between the SP and gpsimd DGE
                    # queues (tail pieces add ACT) so the sequencer's ~1.2us
                    # per-DMA issue cost doesn't serialize the drain.
                    a = QC * c2 + 512 * seg
                    po = (pool or psfl).tile([128, 512], f32, tag=ptag,
                                             name=f"po{c2}{dm}{seg}")
                    for f in range(2):
                        nc.tensor.matmul(
                            po,
                            lhsT=wo_sb[:, f, 128 * dm:128 * (dm + 1)],
                            rhs=attnT[:, f, a:a + 512],
                            start=(f == 0),
                            stop=(f == 1),
                        )
                    ob = opool.tile([128, 512], bf, tag="ob",
                                    name=f"ob{c2}{dm}{seg}")
                    if evac == "act":
                        nc.scalar.copy(ob, po)
                    else:
                        nc.vector.tensor_copy(ob, po)
                    (q or nc.sync).dma_start(
                        outT_d.ap()[128 * dm:128 * (dm + 1), a:a + 512], ob)

                # ---------------- attention unit ----------------
                class Unit:
                    """One (head-pair mi, q-chunk c) attention unit."""

                    def __init__(self, mi, c):
                        self.mi, self.c = mi, c
                        self.q0 = QC * c
                        self.njt = (self.q0 + QC) // KT
                        self.e = epool.tile([128, 2, EW[c]], bf, tag="e",
                                            name=f"e{mi}{c}")
                        self.av4 = None
                        self.av4_idx = 4
                        self.av4_started = False
                        self.off = []
                        o = 0
                        for j in range(self.njt):
                            self.off.append(o)
                            o += QC - max(0, KT * j - self.q0)

                    def step_scores_exp(self, j):
                        """scores + exp for k-tile j. Per 512-col segment,
                        one psum tile [128, 2, seg] holds both heads (one
                        bank each) and ONE activation exps both; the
                        staircase masks the diagonal block (always the first
                        128 cols of segment 0) before exp."""
                        mi, q0 = self.mi, self.q0
                        k0 = KT * j
                        vs = max(0, k0 - q0)
                        for si, (a, b) in enumerate(
                                [(vs, 512), (512, QC)] if vs < 512
                                else [(vs, QC)]):
                            # full [128, 2, 512] tile even for narrow
                            # segments: the head stride must be exactly one
                            # PSUM bank so each head's accumulation group is
                            # bank-private.
                            ps = pssc.tile([128, 2, 512], f32, tag="sc",
                                           name=f"ps{mi}{self.c}_{j}_{si}")
                            diag_here = (k0 >= q0) and (a == vs)
                            for hh in range(2):
                                pb = 64 * hh
                                nc.tensor.matmul(
                                    ps[:, hh, 0:b - a],
                                    lhsT=kT_sb[pb:pb + DK, mi, k0:k0 + KT],
                                    rhs=qT_sb[pb:pb + DK, mi, q0 + a:q0 + b],
                                    start=True,
                                    stop=not diag_here,
                                )
                                if diag_here:
                                    nc.tensor.matmul(
                                        ps[:, hh, 0:KT],
                                        lhsT=stA,
                                        rhs=stB,
                                        start=False,
                                        stop=True,
                                    )
                            o = self.off[j] + a - vs
                            nc.scalar.activation(
                                self.e[:, :, o:o + b - a],
                                ps[:, :, 0:b - a], Exp, scale=0.125)

                    def chain_qb(self, qb, dma_tp=False, act_norm=False):
                        """q-block qb is complete after exp j=qg: run its AV
                        chains, normalize, transpose into attnT. Four chains
                        (2 q-blocks x 2 heads) pack into one PSUM bank as a
                        single accumulation group: the first matmul's start
                        pending-zeroes the bank, later chains' first writes
                        consume it, the last chain's last matmul stops.
                        Transpose into attnT via XBAR DMA (both heads in one
                        [128,128] call, off the PE) except when latency
                        matters (dma_tp=False: PE transpose-mode)."""
                        mi, c, q0 = self.mi, self.c, self.q0
                        qg = NQB * c + qb
                        aq2 = spool.tile([128, 2, DK], bf, tag="aq2",
                                         name=f"aq2_{mi}{c}{qb}", bufs=6)
                        if not dma_tp:
                            tp = psfl.tile([128, 128], bf, tag="fl",
                                           name=f"tp{mi}{c}{qb}")
                        for hh in range(2):
                            h = 2 * mi + hh
                            if self.av4_idx == 4:
                                self.av4 = psav.tile(
                                    [128, 4, DK + 1], f32, tag="av",
                                    name=f"av4_{mi}{c}{qb}")
                                self.av4_idx = 0
                                self.av4_started = False
                            av = self.av4[:, self.av4_idx, :]
                            last_in_group = self.av4_idx == 3
                            self.av4_idx += 1
                            for j in range(qg + 1):
                                vs = max(0, KT * j - q0)
                                col = self.off[j] + 128 * qg - q0 - vs
                                nc.tensor.matmul(
                                    av,
                                    lhsT=self.e[:, hh, col:col + 128],
                                    rhs=v_sb[:, j, h, :],
                                    start=not self.av4_started,
                                    stop=(last_in_group and j == qg),
                                )
                                self.av4_started = True
                            rden = spool.tile([128, 1], f32, tag=f"rden{hh}",
                                              name=f"rden{mi}{c}{qb}{hh}")
                            nc.vector.reciprocal(rden, av[:, DK:DK + 1])
                            if act_norm:
                                nc.scalar.mul(aq2[:, hh, :], av[:, 0:DK], rden)
                            else:
                                nc.vector.tensor_scalar_mul(
                                    aq2[:, hh, :], av[:, 0:DK], rden)
                            if not dma_tp:
                                nc.tensor.transpose(
                                    tp[64 * hh:64 * (hh + 1), :],
                                    aq2[:, hh, :], ident)
                        if dma_tp:
                            nc.sync.dma_start_transpose(
                                attnT[:, mi, KT * qg:KT * (qg + 1)], aq2)
                        else:
                            nc.vector.tensor_copy(
                                attnT[:, mi, KT * qg:KT * (qg + 1)], tp)

                # ---------------- emission schedule ----------------
                import os as _os
                FILLF = float(_os.environ.get('FILLF', '1.3'))
                PE_NS = 1.0 / 2.4  # warm ns per streamed column
                proj_q, done = [], set()

                def piece(key, cols, fn):
                    proj_q.append((key, cols * PE_NS, fn))

                def fill_one():
                    if proj_q:
                        key, cost, fn = proj_q.pop(0)
                        fn()
                        done.add(key)
                        return cost
                    return None

                def fill_ns(budget):
                    while budget > 0 and proj_q:
                        budget -= fill_one()

                def need(*keys):
                    while not all(k in done for k in keys):
                        if fill_one() is None:
                            raise RuntimeError(f"missing pieces {keys}")

                def run_unit(u, needs0=(), needs8=(), inject=None,
                             dma_tp=False, act_norm=False):
                    need(*needs0)
                    u.step_scores_exp(0)
                    for j in range(u.njt):
                        vs = max(0, KT * j - u.q0)
                        w = QC - vs
                        diag = KT * j >= u.q0

                        # budget: cover the exps minus the scores work; the
                        # AV chains are dependency-blocked on exp_j, so they
                        # don't count as guaranteed PE coverage.
                        nseg = 2 if vs < 512 else 1
                        act_j = 2 * 0.833 * w + 185 * nseg
                        pe_j = PE_NS * 2 * (w + (2 * KT if diag else 0))
                        if j + 1 == NQB and needs8:
                            need(*needs8)
                        if j + 1 < u.njt:
                            u.step_scores_exp(j + 1)
                        if diag:
                            qb = j - NQB * u.c
                            need(f"v{j}")
                            u.chain_qb(qb, dma_tp=dma_tp,
                                       act_norm=act_norm)
                        if inject:
                            for fn in inject.pop(j, []):
                                fn()
                        fill_ns((act_j - pe_j) * FILLF)

                # Unit order (0,1), (1,1), (0,0), (1,0): the q-chunk-1 units
                # (long exp chains) run while projection filler is plentiful;
                # the final unit is a cheap c=0 one, so the tail after the
                # last exp is short and wo chunk 1 becomes mid-kernel filler.

                # startup: six projection pieces chase the x-chunk DMAs,
                # parked on the otherwise-idle attention psum banks.
                proj_qk_piece(0, 1, "q", 0)
                proj_qk_piece(0, 1, "q", 1)
                done.update({"q01a", "q01b"})

                def k_piece(seg):  # k(m=0) chunk 0, on the av banks
                    a = 512 * seg
                    ps = psav.tile([128, 512], f32, tag="av", name=f"pk{seg}")
                    for kc in range(KC):
                        nc.tensor.matmul(
                            ps,
                            lhsT=wk_sb[:, kc, 0:128],
                            rhs=xT_sb[:, kc, a:a + 512],
                            start=(kc == 0),
                            stop=(kc == KC - 1),
                        )
                    nc.vector.tensor_copy(kT_sb[:, 0, a:a + 512], ps)

                k_piece(0)
                k_piece(1)
                done.update({"k00a", "k00b"})
                # q00 pieces can chase the x stream (wq is resident); v
                # tiles can't (wv lands after the last x chunk).
                proj_qk_piece(0, 0, "q", 0, pool=pssc, ptag="sc")
                proj_qk_piece(0, 0, "q", 1, pool=pssc, ptag="sc")
                done.update({"q00a", "q00b"})
                piece("v0", 2048, lambda: proj_v(0))
                piece("v1", 2048, lambda: proj_v(1))

                def qk_pieces(nm, m, c2):
                    for sg, sn in ((0, "a"), (1, "b")):
                        piece(f"{nm}{m}{c2}{sn}", 4096,
                              lambda nm=nm, m=m, c2=c2, sg=sg:
                              proj_qk_piece(m, c2, nm, sg))

                for st in range(2, NQB):
                    piece(f"v{st}", 2048, lambda st=st: proj_v(st))
                qk_pieces("k", 0, 1)
                for st in range(NQB, 12):
                    piece(f"v{st}", 2048, lambda st=st: proj_v(st))
                qk_pieces("q", 1, 1)
                qk_pieces("k", 1, 0)
                for st in range(12, 16):
                    piece(f"v{st}", 2048, lambda st=st: proj_v(st))
                qk_pieces("k", 1, 1)
                qk_pieces("q", 1, 0)

                run_unit(Unit(0, 1),
                         needs0=("q01a", "q01b", "k00a", "k00b"),
                         needs8=("k01a", "k01b"))
                # wo chunk-1 seg-0 only needs q-blocks 8..11 of both c=1
                # units: inject into the tail of the second unit.
                inject = {12 + i: [
                    (lambda dm=dm: wo_piece(1, dm, 0)) for dm in (2 * i, 2 * i + 1)
                ] for i in range(4)}
                run_unit(Unit(1, 1),
                         needs0=("q11a", "q11b", "k10a", "k10b"),
                         needs8=("k11a", "k11b"),
                         inject=inject)

                for dm in range(8):
                    piece(f"wo1{dm}1", 1024,
                          lambda dm=dm: wo_piece(1, dm, 1))

                run_unit(Unit(0, 0))
                inject = {4 + i: [
                    (lambda dm=dm: wo_piece(0, dm, 0)) for dm in (2 * i, 2 * i + 1)
                ] for i in range(4)}
                run_unit(Unit(1, 0), needs0=("q10a", "q10b"), inject=inject)
                while fill_one() is not None:
                    pass

                # tail: the attention banks and ScalarE are idle now -
                # rotate psum pools and evac engines, and pair up the final
                # out-DMAs (two 128-row slabs per transfer) so the HWDGE +
                # DMA-semaphore pipeline drains in half the steps.
                a1 = 512  # chunk 0, segment 1
                out_pair_view = outT_d.ap().rearrange(
                    "(dmp two p) c -> dmp p two c", two=2, p=128)
                for dmp in range(4):
                    ob2 = opool.tile([128, 2, 512], bf, tag="ob2",
                                     name=f"ob2_{dmp}", bufs=3)
                    for half in range(2):
                        dm = 2 * dmp + half
                        pool, ptag = [(psav, "av"), (pssc, "sc")][dm % 2]
                        po = pool.tile([128, 512], f32, tag=ptag,
                                       name=f"po0{dm}1")
                        for f in range(2):
                            nc.tensor.matmul(
                                po,
                                lhsT=wo_sb[:, f, 128 * dm:128 * (dm + 1)],
                                rhs=attnT[:, f, a1:a1 + 512],
                                start=(f == 0),
                                stop=(f == 1),
                            )
                        if dm % 2:
                            nc.scalar.copy(ob2[:, half, :], po)
                        else:
                            nc.vector.tensor_copy(ob2[:, half, :], po)
                    nc.sync.dma_start(
                        out_pair_view[dmp, :, :, a1:a1 + 512], ob2)

    nc.compile()
    return nc


def _get_nc():
    if "nc" not in _CACHE:
        _CACHE["nc"] = _build_nc()
    return _CACHE["nc"]


def _consts():
    import ml_dtypes

    t = np.arange(128)
    ident = np.eye(128).astype(ml_dtypes.bfloat16)
    stA = (t[:, None] <= t[None, :]).astype(ml_dtypes.bfloat16)
    stB = np.where(t[:, None] > t[None, :], -240.0, 0.0).astype(ml_dtypes.bfloat16)
    return ident, stA, stB


def _make_in_maps(x, wq, wk, wv, wo):
    import ml_dtypes

    bf = ml_dtypes.bfloat16
    ident, stA, stB = _consts()
    x = np.asarray(x, np.float32)
    xTs = [np.ascontiguousarray(x[b].T).astype(bf) for b in range(B)]
    wqb = np.asarray(wq, np.float32).astype(bf)
    wkb = np.asarray(wk, np.float32).astype(bf)
    wvb = np.asarray(wv, np.float32).astype(bf)
    wob = np.asarray(wo, np.float32).astype(bf)
    in_maps = []
    for c in range(NCORES):
        b, g = divmod(c, HPC)
        cols = slice(g * GW, (g + 1) * GW)
        in_maps.append({
            "xT": xTs[b],
            "wq": np.ascontiguousarray(wqb[:, cols]),
            "wk": np.ascontiguousarray(wkb[:, cols]),
            "wv": np.ascontiguousarray(wvb[:, cols]),
            "wo": np.ascontiguousarray(wob[cols, :]),
            "ident": ident,
            "stairA": stA,
            "stairB": stB,
        })
    return in_maps


def run(x, wq, wk, wv, wo, trace=False):
    from concourse.bass_utils import run_bass_kernel_spmd

    nc = _get_nc()
    in_maps = _make_in_maps(x, wq, wk, wv, wo)
    res = run_bass_kernel_spmd(nc, in_maps, list(range(NCORES)), trace=trace)
    acc = np.zeros((B, D, S), np.float64)
    for c in range(NCORES):
        acc[c // HPC] += res.results[c]["outT"]
    out = np.ascontiguousarray(acc.transpose(0, 2, 1).astype(np.float32))
    return out, res


def kernel(x, wq, wk, wv, wo):
    out, _ = run(x, wq, wk, wv, wo, trace=False)
    return out
